# revision 26
# baseline (speedup 1.0000x reference)
"""GQA cross-attention kernel for Trainium2 (8 NeuronCores, Bass/Tile).

Problem: q (2,2048,16,64) f32, kv (2,2048,2,4,64) f32, key_padding_mask (2,2048)
bool.  Reference: GQA attention with additive -10000 padding bias and a causal
mask shifted by the per-batch valid key count sk, softmax over keys.

Key observations used here:
  * Every padded key position is also causal-masked, so only the shifted-causal
    structure matters.  With u := q_idx - (2048 - sk) the valid region is the
    causal triangle u >= k over the first sk keys; the shift is applied on the
    HOST when laying out Q^T per core, so the device program is a static causal
    flash-attention kernel.
  * Rows q_idx < 2048 - sk have no valid key: reference softmaxes equal
    -10000s -> uniform weights -> output = mean over all 2048 v rows.  Pure
    host-side fixup.
  * exp without max-subtraction is safe (|score*0.125| <~ 8); the softmax
    denominator comes from a ones-column appended to V (PV matmul yields
    [num | den]); the division happens on host.

Device program (per core, 4 head-instances = 2 heads x 2 batches):
  u-chunk-outer loop (512-wide output chunks, [65,512] single-bank fp32 PSUM
  accumulators, two chunks live so TWO INSTANCES are processed interleaved --
  the PE always has an independent stream while the other instance's exp
  catches up).  128-key score blocks are processed in PAIRS sharing one
  u-stream window: the PE row-tiles the two QK matmuls at row groups
  0-63 / 64-127 (K^T and Q^T are host-duplicated into both SBUF partition
  halves), producing TWO full [128 x w] score strips in w cycles -- 2x QK
  throughput vs one matmul.  Per strip: exp on ScalarE (or a Schraudolph
  fast-exp2 on VectorE for a third of the strips, balancing the two
  engines), triangle mask on the diagonal block, then
  [num|den] += V'.T @ P^T (PV emission delayed one pair so the tensor queue
  never stalls on the activation).  Six single-bank score tiles give the PE
  ~3 strips of dependency runway toward the warm HAM clock.
  Startup: critical-path-first DMAs and exp-table preload during the initial
  DMA wait.  Output DMA'd as fp16.
"""

import os
import ml_dtypes
import numpy as np

F16 = np.float16

import concourse.bass as bass
import concourse.mybir as mybir
import concourse.tile as tile
from concourse import bacc
from concourse.bass_utils import run_bass_kernel_spmd

B, SQ, SK, H, HK, D = 2, 2048, 2048, 16, 4, 64
NCORES = 8
P = 128
FP = mybir.dt.float32
FR = mybir.dt.float16
I16 = mybir.dt.int16
CHW = 512   # u-chunk width (1 PSUM bank of fp32)

# Schraudolph fast-exp2 constants for fp16 bit-pattern exp(0.125*s):
# i16 = trunc(s*EXPA + EXPB); bitcast fp16.  max rel err ~3.0%, mean +1%.
EXPA = 0.125 * 1.4426950408889634 * 1024.0
EXPB = 15315.5
DVE_EXP_PERIOD = 3  # every 3rd strip exp'd on VectorE (0 = disabled)
WARMUP_MM = 0       # junk matmuls at t0 to release the PE HAM clock gate

LAST_EXEC_NS = None


def _ceil_div(a, b):
    return -(-a // b)


def _build_program(sks):
    """Build + compile the SPMD program for per-batch valid key counts sks."""
    nc = bacc.Bacc("TRN2", target_bir_lowering=False, debug=False,
                   num_devices=NCORES)

    # q/k duplicated into both partition halves on host: [128, S]
    qT_d = nc.dram_tensor("qT", [4, P, SQ], FR, kind="ExternalInput").ap()
    kT_d = nc.dram_tensor("kT", [B, P, SK], FR, kind="ExternalInput").ap()
    vp_d = nc.dram_tensor("vp", [B, P, (SK // P) * 65], FR,
                          kind="ExternalInput").ap()
    tri_d = nc.dram_tensor("tri", [P, P], FR, kind="ExternalInput").ap()
    out_d = nc.dram_tensor("outT", [4, 65, SQ], FR, kind="ExternalOutput").ap()

    EXP = mybir.ActivationFunctionType.Exp
    MUL = mybir.AluOpType.mult
    ADD = mybir.AluOpType.add

    with tile.TileContext(nc) as tc:
        with (
            tc.tile_pool(name="const", bufs=1) as cpool,
            tc.tile_pool(name="kv", bufs=1) as kvpool,
            tc.tile_pool(name="qin", bufs=4) as qpool,
            tc.tile_pool(name="pt", bufs=10) as ppool,
            tc.tile_pool(name="oc", bufs=3) as opool,
            tc.tile_pool(name="ps", bufs=6, space="PSUM") as spool,
            tc.tile_pool(name="pa", bufs=2, space="PSUM") as apool,
        ):
            kT_sb = []
            vp_sb = []
            for b in range(B):
                kt_t = kvpool.tile([P, SK], FR, name=f"kT{b}", tag=f"kT{b}")
                kT_sb.append(kt_t)
                vp_t = kvpool.tile([P, (SK // P) * 65], FR, name=f"vp{b}",
                                   tag=f"vp{b}")
                vp_sb.append(vp_t)
            tri_sb = cpool.tile([P, P], FR, name="tri_sb")
            q_sb = [qpool.tile([P, SQ], FR, name=f"q{j}", tag=f"q{j}")
                    for j in range(4)]

            # --- input DMAs, critical path first; the two transfers the
            # first matmul pair needs go on the otherwise-idle Activation
            # HWDGE ring so they complete in parallel with the Sync ring ---
            nc.scalar.dma_start(q_sb[0][:, 0:512], qT_d[0][:, 0:512])
            nc.scalar.dma_start(kT_sb[0][:, 0:256], kT_d[0][:, 0:256])
            # exp-table preload during the DMA wait
            dmy = cpool.tile([P, 32], FR, name="dmy")
            nc.vector.memset(dmy[:, 0:16], 0.0)
            nc.scalar.activation(dmy[:, 16:32], dmy[:, 0:16], EXP, scale=1.0)
            nc.sync.dma_start(tri_sb[:], tri_d[:])
            nc.sync.dma_start(q_sb[1][:, 0:512], qT_d[1][:, 0:512])
            nc.sync.dma_start(kT_sb[0][:, 256:1024], kT_d[0][:, 256:1024])
            nc.sync.dma_start(vp_sb[0][:, 0:130], vp_d[0][:, 0:130])
            nc.sync.dma_start(q_sb[0][:, 512:1024], qT_d[0][:, 512:1024])
            nc.sync.dma_start(q_sb[1][:, 512:1024], qT_d[1][:, 512:1024])
            nc.sync.dma_start(vp_sb[0][:, 130:1040], vp_d[0][:, 130:1040])
            nc.sync.dma_start(q_sb[0][:, 1024:2048], qT_d[0][:, 1024:2048])
            nc.sync.dma_start(q_sb[1][:, 1024:2048], qT_d[1][:, 1024:2048])
            nc.sync.dma_start(kT_sb[0][:, 1024:2048], kT_d[0][:, 1024:2048])
            nc.sync.dma_start(q_sb[2][:], qT_d[2])
            nc.sync.dma_start(kT_sb[1][:], kT_d[1])
            nc.sync.dma_start(vp_sb[1][:], vp_d[1])
            nc.sync.dma_start(q_sb[3][:], qT_d[3])

            # --- PE clock warm-up: junk matmuls as soon as q piece 0 lands ---
            if WARMUP_MM:
                wps = spool.tile([P, 512], FP, name="ps", tag="ps")
                for i in range(WARMUP_MM):
                    nc.tensor.matmul(
                        wps[:, 0:512], lhsT=q_sb[0][0:64, 0:128],
                        rhs=q_sb[0][0:64, 0:512],
                        start=True, stop=True, skip_group_check=True)

            nstrip = [0]  # round-robin counter for DVE exp offload

            def inst_work(j):
                b = 0 if j < 2 else 1
                U = sks[b]
                KT = _ceil_div(U, P)
                NCH = _ceil_div(U, CHW)
                for c in range(NCH):
                    cs = CHW * c
                    ce = min(U, CHW * (c + 1))
                    acc = apool.tile([65, CHW], FP, name="acc", tag="acc")
                    kt_hi = min(KT, _ceil_div(ce, P))
                    stop = kt_hi - 1

                    # delayed PV work: (kt, kw, m0, e0, pt_ap)
                    pending = []

                    def flush_pv():
                        for (fkt, fkw, fm0, fe0, fpt) in pending:
                            nc.tensor.matmul(
                                acc[:, fe0 - cs:ce - cs],
                                lhsT=vp_sb[b][0:fkw,
                                              65 * fkt:65 * (fkt + 1)],
                                rhs=fpt[0:fkw, fe0 - fm0:ce - fm0],
                                start=(fkt == 0), stop=(fkt == stop),
                                skip_group_check=True,
                            )
                        pending.clear()

                    for t in range(_ceil_div(kt_hi, 2)):
                        blocks = [2 * t]
                        if 2 * t + 1 < kt_hi:
                            blocks.append(2 * t + 1)
                        k0A = P * 2 * t
                        m0 = max(cs, k0A)  # shared stream window [m0, ce)
                        # QK: both blocks stream the window, row-tiled at PE
                        # row groups 0-63 / 64-127 (concurrent)
                        ptile = []
                        for bi, kt in enumerate(blocks):
                            k0 = P * kt
                            kw = min(P, U - k0)
                            hf = 64 * bi
                            ps = spool.tile([P, CHW], FP, name="ps",
                                            tag="ps")
                            ptile.append(ps)
                            nc.tensor.matmul(
                                ps[0:kw, 0:ce - m0],
                                lhsT=kT_sb[b][hf:hf + 64, k0:k0 + kw],
                                rhs=q_sb[j][hf:hf + 64, m0:ce],
                                start=True, stop=True,
                                skip_group_check=True,
                            )

                        newpv = []
                        for bi, kt in enumerate(blocks):
                            k0 = P * kt
                            kw = min(P, U - k0)
                            e0 = max(cs, k0)  # strip's own valid start
                            ps = ptile[bi]
                            use_dve = (DVE_EXP_PERIOD and
                                       nstrip[0] % DVE_EXP_PERIOD ==
                                       DVE_EXP_PERIOD - 1)
                            nstrip[0] += 1
                            if use_dve:
                                pti = ppool.tile([P, CHW], I16,
                                                 name="pt", tag="pt")
                                nc.vector.tensor_scalar(
                                    pti[0:kw, e0 - m0:ce - m0],
                                    ps[0:kw, e0 - m0:ce - m0],
                                    EXPA, EXPB, MUL, ADD)
                                pt = pti.bitcast(FR)
                            else:
                                pt = ppool.tile([P, CHW], FR,
                                                name="pt", tag="pt")
                                nc.scalar.activation(
                                    pt[0:kw, e0 - m0:ce - m0],
                                    ps[0:kw, e0 - m0:ce - m0],
                                    EXP, scale=0.125)
                            # causal triangle mask on the diagonal block
                            d0 = max(k0, e0)
                            d1 = min(k0 + P, ce)
                            if k0 >= cs and d0 < d1:
                                nc.vector.tensor_mul(
                                    pt[0:kw, d0 - m0:d1 - m0],
                                    pt[0:kw, d0 - m0:d1 - m0],
                                    tri_sb[0:kw, d0 - k0:d1 - k0])
                            newpv.append((kt, kw, m0, e0, pt))

                        flush_pv()
                        pending.extend(newpv)
                        yield

                    flush_pv()
                    cw = ce - cs
                    oc = opool.tile([65, CHW], FR, name="oc", tag="oc")
                    nc.vector.tensor_copy(oc[:, 0:cw], acc[:, 0:cw])
                    nc.sync.dma_start(out_d[j, :, cs:cs + cw], oc[:, 0:cw])
                    yield

            # interleave two instances at a time so the PE always has an
            # independent stream to chew on while the other's exp catches up
            for pair in ((0, 1), (2, 3)):
                gens = [inst_work(pair[0]), inst_work(pair[1])]
                done = [False, False]
                i = 0
                while not all(done):
                    if not done[i]:
                        try:
                            next(gens[i])
                        except StopIteration:
                            done[i] = True
                    i ^= 1

    nc.compile()
    return nc


_prog_cache = {}


def _get_program(sks):
    if sks not in _prog_cache:
        _prog_cache[sks] = _build_program(sks)
    return _prog_cache[sks]


def kernel(q, kv, key_padding_mask):
    global LAST_EXEC_NS
    q = np.asarray(q, dtype=np.float32)
    kv = np.asarray(kv, dtype=np.float32)
    mask = np.asarray(key_padding_mask)

    sk = mask.sum(axis=1).astype(np.int64)  # (B,) valid key counts
    c = (SQ - sk).astype(np.int64)
    prog = _get_program((int(sk[0]), int(sk[1])))

    k_all = kv[:, :, 0]  # (B, SK, HK, D)
    v_all = kv[:, :, 1]

    tri = (np.arange(P)[None, :] >= np.arange(P)[:, None]).astype(np.float32)

    kT_by_g = {}
    vp_by_g = {}
    for g in range(HK):
        kT = k_all[:, :, g, :].transpose(0, 2, 1)  # (B, D, SK)
        kT_by_g[g] = np.ascontiguousarray(
            np.concatenate([kT, kT], axis=1))  # (B, 128, SK) duplicated
        vpz = np.ones((B, SK, 65), dtype=np.float32)
        vpz[:, :, :64] = v_all[:, :, g, :]
        vp = vpz.reshape(B, SK // P, P, 65).transpose(0, 2, 1, 3)
        vp_by_g[g] = np.ascontiguousarray(vp.reshape(B, P, (SK // P) * 65))

    def core_instances(core):
        g = core // 2
        hp = core % 2
        h0 = 4 * g + 2 * hp
        return g, [(0, h0), (0, h0 + 1), (1, h0), (1, h0 + 1)]

    in_maps = []
    for core in range(NCORES):
        g, insts = core_instances(core)
        qT = np.zeros((4, P, SQ), dtype=np.float32)
        for jj, (b, h) in enumerate(insts):
            U = int(sk[b])
            qh = q[b, c[b]:, h, :].T  # (64, U)
            qT[jj, 0:64, :U] = qh
            qT[jj, 64:128, :U] = qh
        in_maps.append({
            "qT": qT.astype(F16),
            "kT": kT_by_g[g].astype(F16),
            "vp": vp_by_g[g].astype(F16),
            "tri": tri.astype(F16),
        })

    trace = bool(os.environ.get("BASS_KERNEL_TRACE"))
    res = run_bass_kernel_spmd(prog, in_maps, list(range(NCORES)),
                               trace=trace)
    LAST_EXEC_NS = res.exec_time_ns

    out = np.empty((B, SQ, H, D), dtype=np.float32)
    # fully-masked rows: uniform softmax over all SK keys -> mean of v
    vmean = v_all.mean(axis=1)  # (B, HK, D)
    for b in range(B):
        if c[b] > 0:
            for g in range(HK):
                for h in range(4 * g, 4 * g + 4):
                    out[b, :c[b], h, :] = vmean[b, g]

    for core in range(NCORES):
        g, insts = core_instances(core)
        o = np.asarray(res.results[core]["outT"], dtype=np.float32)
        for jj, (b, h) in enumerate(insts):
            U = int(sk[b])
            num = o[jj, :64, :U]
            den = o[jj, 64, :U]
            out[b, c[b]:, h, :] = (num / den[None, :]).T

    return out


# revision 27
# speedup vs baseline: 1.1948x; 1.1948x over previous
"""GQA cross-attention kernel for Trainium2 (8 NeuronCores, Bass/Tile).

Problem: q (2,2048,16,64) f32, kv (2,2048,2,4,64) f32, key_padding_mask (2,2048)
bool.  Reference: GQA attention with additive -10000 padding bias and a causal
mask shifted by the per-batch valid key count sk, softmax over keys.

Key observations used here:
  * Every padded key position is also causal-masked, so only the shifted-causal
    structure matters.  With u := q_idx - (2048 - sk) the valid region is the
    causal triangle u >= k over the first sk keys; the shift is applied on the
    HOST when laying out Q^T per core, so the device program is a static causal
    flash-attention kernel.
  * Rows q_idx < 2048 - sk have no valid key: reference softmaxes equal
    -10000s -> uniform weights -> output = mean over all 2048 v rows.  Pure
    host-side fixup.
  * exp without max-subtraction is safe (|score*0.125| <~ 8); the softmax
    denominator comes from a ones-column appended to V (PV matmul yields
    [num | den]); the division happens on host.

Device program (per core, 4 head-instances = 2 heads x 2 batches):
  u-chunk-outer loop (512-wide output chunks, [65,512] single-bank fp32 PSUM
  accumulators, two chunks live so TWO INSTANCES are processed interleaved --
  the PE always has an independent stream while the other instance's exp
  catches up).  128-key score blocks are processed in PAIRS sharing one
  u-stream window: the PE row-tiles the two QK matmuls at row groups
  0-63 / 64-127 (K^T and Q^T are host-duplicated into both SBUF partition
  halves), producing TWO full [128 x w] score strips in w cycles -- 2x QK
  throughput vs one matmul.  Per strip: exp on ScalarE (or a Schraudolph
  fast-exp2 on VectorE for a third of the strips, balancing the two
  engines), triangle mask on the diagonal block, then
  [num|den] += V'.T @ P^T (PV emission delayed one pair so the tensor queue
  never stalls on the activation).  Six single-bank score tiles give the PE
  ~3 strips of dependency runway toward the warm HAM clock.
  Startup: critical-path-first DMAs and exp-table preload during the initial
  DMA wait.  Output DMA'd as fp16.
"""

import os
import ml_dtypes
import numpy as np

F16 = np.float16

import concourse.bass as bass
import concourse.mybir as mybir
import concourse.tile as tile
from concourse import bacc
from concourse.bass_utils import run_bass_kernel_spmd

B, SQ, SK, H, HK, D = 2, 2048, 2048, 16, 4, 64
NCORES = 8
P = 128
FP = mybir.dt.float32
FR = mybir.dt.float16
I16 = mybir.dt.int16
CHW = 512   # u-chunk width (1 PSUM bank of fp32)

# Schraudolph fast-exp2 constants for fp16 bit-pattern exp(0.125*s):
# i16 = trunc(s*EXPA + EXPB); bitcast fp16.  max rel err ~3.0%, mean +1%.
EXPA = 0.125 * 1.4426950408889634 * 1024.0
EXPB = 15315.5
DVE_EXP_PERIOD = 3  # every 3rd strip exp'd on VectorE (0 = disabled)
WARMUP_MM = 0       # junk matmuls at t0 to release the PE HAM clock gate

LAST_EXEC_NS = None


def _ceil_div(a, b):
    return -(-a // b)


def _build_program(sks):
    """Build + compile the SPMD program for per-batch valid key counts sks."""
    nc = bacc.Bacc("TRN2", target_bir_lowering=False, debug=False,
                   num_devices=NCORES)

    # q/k duplicated into both partition halves on host: [128, S]
    qT_d = nc.dram_tensor("qT", [4, P, SQ], FR, kind="ExternalInput").ap()
    kT_d = nc.dram_tensor("kT", [B, P, SK], FR, kind="ExternalInput").ap()
    vp_d = nc.dram_tensor("vp", [B, P, (SK // P) * 65], FR,
                          kind="ExternalInput").ap()
    tri_d = nc.dram_tensor("tri", [P, P], FR, kind="ExternalInput").ap()
    out_d = nc.dram_tensor("outT", [4, 65, SQ], FR, kind="ExternalOutput").ap()

    EXP = mybir.ActivationFunctionType.Exp
    MUL = mybir.AluOpType.mult
    ADD = mybir.AluOpType.add

    with tile.TileContext(nc) as tc:
        with (
            tc.tile_pool(name="const", bufs=1) as cpool,
            tc.tile_pool(name="kv", bufs=1) as kvpool,
            tc.tile_pool(name="qin", bufs=4) as qpool,
            tc.tile_pool(name="pt", bufs=10) as ppool,
            tc.tile_pool(name="oc", bufs=3) as opool,
            tc.tile_pool(name="ps", bufs=6, space="PSUM") as spool,
            tc.tile_pool(name="pa", bufs=2, space="PSUM") as apool,
        ):
            kT_sb = []
            vp_sb = []
            for b in range(B):
                kt_t = kvpool.tile([P, SK], FR, name=f"kT{b}", tag=f"kT{b}")
                kT_sb.append(kt_t)
                vp_t = kvpool.tile([P, (SK // P) * 65], FR, name=f"vp{b}",
                                   tag=f"vp{b}")
                vp_sb.append(vp_t)
            tri_sb = cpool.tile([P, P], FR, name="tri_sb")
            q_sb = [qpool.tile([P, SQ], FR, name=f"q{j}", tag=f"q{j}")
                    for j in range(4)]

            # --- input DMAs, critical path first; the two transfers the
            # first matmul pair needs go on the otherwise-idle Activation
            # HWDGE ring so they complete in parallel with the Sync ring ---
            nc.scalar.dma_start(q_sb[0][:, 0:512], qT_d[0][:, 0:512])
            nc.scalar.dma_start(kT_sb[0][:, 0:256], kT_d[0][:, 0:256])
            # exp-table preload during the DMA wait
            dmy = cpool.tile([P, 32], FR, name="dmy")
            nc.vector.memset(dmy[:, 0:16], 0.0)
            nc.scalar.activation(dmy[:, 16:32], dmy[:, 0:16], EXP, scale=1.0)
            nc.sync.dma_start(tri_sb[:], tri_d[:])
            nc.sync.dma_start(q_sb[1][:, 0:512], qT_d[1][:, 0:512])
            nc.sync.dma_start(vp_sb[0][:, 0:130], vp_d[0][:, 0:130])
            nc.sync.dma_start(q_sb[0][:, 512:1024], qT_d[0][:, 512:1024])
            nc.sync.dma_start(kT_sb[0][:, 256:1024], kT_d[0][:, 256:1024])
            nc.sync.dma_start(q_sb[1][:, 512:1024], qT_d[1][:, 512:1024])
            nc.sync.dma_start(vp_sb[0][:, 130:1040], vp_d[0][:, 130:1040])
            nc.sync.dma_start(q_sb[0][:, 1024:2048], qT_d[0][:, 1024:2048])
            nc.sync.dma_start(q_sb[1][:, 1024:2048], qT_d[1][:, 1024:2048])
            nc.sync.dma_start(kT_sb[0][:, 1024:2048], kT_d[0][:, 1024:2048])
            nc.sync.dma_start(q_sb[2][:], qT_d[2])
            nc.sync.dma_start(kT_sb[1][:], kT_d[1])
            nc.sync.dma_start(vp_sb[1][:], vp_d[1])
            nc.sync.dma_start(q_sb[3][:], qT_d[3])

            # --- PE clock warm-up: junk matmuls as soon as q piece 0 lands ---
            if WARMUP_MM:
                wps = spool.tile([P, 512], FP, name="ps", tag="ps")
                for i in range(WARMUP_MM):
                    nc.tensor.matmul(
                        wps[:, 0:512], lhsT=q_sb[0][0:64, 0:128],
                        rhs=q_sb[0][0:64, 0:512],
                        start=True, stop=True, skip_group_check=True)

            nstrip = [0]  # round-robin counter for DVE exp offload

            def inst_work(j):
                b = 0 if j < 2 else 1
                U = sks[b]
                KT = _ceil_div(U, P)
                NCH = _ceil_div(U, CHW)
                for c in range(NCH):
                    cs = CHW * c
                    ce = min(U, CHW * (c + 1))
                    acc = apool.tile([65, CHW], FP, name="acc", tag="acc")
                    kt_hi = min(KT, _ceil_div(ce, P))
                    stop = kt_hi - 1

                    # delayed PV work: batches of (kt, kw, m0, e0, pt_ap)
                    pending = []

                    def flush_pv(keep=0):
                        while len(pending) > keep:
                            batch = pending.pop(0)
                            for (fkt, fkw, fm0, fe0, fpt) in batch:
                                nc.tensor.matmul(
                                    acc[:, fe0 - cs:ce - cs],
                                    lhsT=vp_sb[b][0:fkw,
                                                  65 * fkt:65 * (fkt + 1)],
                                    rhs=fpt[0:fkw, fe0 - fm0:ce - fm0],
                                    start=(fkt == 0), stop=(fkt == stop),
                                    skip_group_check=True,
                                )

                    for t in range(_ceil_div(kt_hi, 2)):
                        blocks = [2 * t]
                        if 2 * t + 1 < kt_hi:
                            blocks.append(2 * t + 1)
                        k0A = P * 2 * t
                        m0 = max(cs, k0A)  # shared stream window [m0, ce)
                        # QK: both blocks stream the window, row-tiled at PE
                        # row groups 0-63 / 64-127 (concurrent)
                        ptile = []
                        for bi, kt in enumerate(blocks):
                            k0 = P * kt
                            kw = min(P, U - k0)
                            hf = 64 * bi
                            ps = spool.tile([P, CHW], FP, name="ps",
                                            tag="ps")
                            ptile.append(ps)
                            nc.tensor.matmul(
                                ps[0:kw, 0:ce - m0],
                                lhsT=kT_sb[b][hf:hf + 64, k0:k0 + kw],
                                rhs=q_sb[j][hf:hf + 64, m0:ce],
                                start=True, stop=True,
                                skip_group_check=True,
                            )

                        newpv = []
                        for bi, kt in enumerate(blocks):
                            k0 = P * kt
                            kw = min(P, U - k0)
                            e0 = max(cs, k0)  # strip's own valid start
                            ps = ptile[bi]
                            use_dve = (DVE_EXP_PERIOD and
                                       nstrip[0] % DVE_EXP_PERIOD ==
                                       DVE_EXP_PERIOD - 1)
                            nstrip[0] += 1
                            if use_dve:
                                pti = ppool.tile([P, CHW], I16,
                                                 name="pt", tag="pt")
                                nc.vector.tensor_scalar(
                                    pti[0:kw, e0 - m0:ce - m0],
                                    ps[0:kw, e0 - m0:ce - m0],
                                    EXPA, EXPB, MUL, ADD)
                                pt = pti.bitcast(FR)
                            else:
                                pt = ppool.tile([P, CHW], FR,
                                                name="pt", tag="pt")
                                nc.scalar.activation(
                                    pt[0:kw, e0 - m0:ce - m0],
                                    ps[0:kw, e0 - m0:ce - m0],
                                    EXP, scale=0.125)
                            # causal triangle mask on the diagonal block
                            d0 = max(k0, e0)
                            d1 = min(k0 + P, ce)
                            if k0 >= cs and d0 < d1:
                                nc.vector.tensor_mul(
                                    pt[0:kw, d0 - m0:d1 - m0],
                                    pt[0:kw, d0 - m0:d1 - m0],
                                    tri_sb[0:kw, d0 - k0:d1 - k0])
                            newpv.append((kt, kw, m0, e0, pt))

                        flush_pv(keep=1)
                        pending.append(newpv)
                        yield

                    flush_pv()
                    cw = ce - cs
                    oc = opool.tile([65, CHW], FR, name="oc", tag="oc")
                    nc.vector.tensor_copy(oc[:, 0:cw], acc[:, 0:cw])
                    nc.sync.dma_start(out_d[j, :, cs:cs + cw], oc[:, 0:cw])
                    yield

            # interleave two instances at a time so the PE always has an
            # independent stream to chew on while the other's exp catches up
            for pair in ((0, 1), (2, 3)):
                gens = [inst_work(pair[0]), inst_work(pair[1])]
                done = [False, False]
                i = 0
                while not all(done):
                    if not done[i]:
                        try:
                            next(gens[i])
                        except StopIteration:
                            done[i] = True
                    i ^= 1

    nc.compile()
    return nc


_prog_cache = {}


def _get_program(sks):
    if sks not in _prog_cache:
        _prog_cache[sks] = _build_program(sks)
    return _prog_cache[sks]


def kernel(q, kv, key_padding_mask):
    global LAST_EXEC_NS
    q = np.asarray(q, dtype=np.float32)
    kv = np.asarray(kv, dtype=np.float32)
    mask = np.asarray(key_padding_mask)

    sk = mask.sum(axis=1).astype(np.int64)  # (B,) valid key counts
    c = (SQ - sk).astype(np.int64)
    prog = _get_program((int(sk[0]), int(sk[1])))

    k_all = kv[:, :, 0]  # (B, SK, HK, D)
    v_all = kv[:, :, 1]

    tri = (np.arange(P)[None, :] >= np.arange(P)[:, None]).astype(np.float32)

    kT_by_g = {}
    vp_by_g = {}
    for g in range(HK):
        kT = k_all[:, :, g, :].transpose(0, 2, 1)  # (B, D, SK)
        kT_by_g[g] = np.ascontiguousarray(
            np.concatenate([kT, kT], axis=1))  # (B, 128, SK) duplicated
        vpz = np.ones((B, SK, 65), dtype=np.float32)
        vpz[:, :, :64] = v_all[:, :, g, :]
        vp = vpz.reshape(B, SK // P, P, 65).transpose(0, 2, 1, 3)
        vp_by_g[g] = np.ascontiguousarray(vp.reshape(B, P, (SK // P) * 65))

    def core_instances(core):
        g = core // 2
        hp = core % 2
        h0 = 4 * g + 2 * hp
        return g, [(0, h0), (0, h0 + 1), (1, h0), (1, h0 + 1)]

    in_maps = []
    for core in range(NCORES):
        g, insts = core_instances(core)
        qT = np.zeros((4, P, SQ), dtype=np.float32)
        for jj, (b, h) in enumerate(insts):
            U = int(sk[b])
            qh = q[b, c[b]:, h, :].T  # (64, U)
            qT[jj, 0:64, :U] = qh
            qT[jj, 64:128, :U] = qh
        in_maps.append({
            "qT": qT.astype(F16),
            "kT": kT_by_g[g].astype(F16),
            "vp": vp_by_g[g].astype(F16),
            "tri": tri.astype(F16),
        })

    trace = bool(os.environ.get("BASS_KERNEL_TRACE"))
    res = run_bass_kernel_spmd(prog, in_maps, list(range(NCORES)),
                               trace=trace)
    LAST_EXEC_NS = res.exec_time_ns

    out = np.empty((B, SQ, H, D), dtype=np.float32)
    # fully-masked rows: uniform softmax over all SK keys -> mean of v
    vmean = v_all.mean(axis=1)  # (B, HK, D)
    for b in range(B):
        if c[b] > 0:
            for g in range(HK):
                for h in range(4 * g, 4 * g + 4):
                    out[b, :c[b], h, :] = vmean[b, g]

    for core in range(NCORES):
        g, insts = core_instances(core)
        o = np.asarray(res.results[core]["outT"], dtype=np.float32)
        for jj, (b, h) in enumerate(insts):
            U = int(sk[b])
            num = o[jj, :64, :U]
            den = o[jj, 64, :U]
            out[b, c[b]:, h, :] = (num / den[None, :]).T

    return out


# revision 28
# speedup vs baseline: 1.2374x; 1.0357x over previous
"""GQA cross-attention kernel for Trainium2 (8 NeuronCores, Bass/Tile).

Problem: q (2,2048,16,64) f32, kv (2,2048,2,4,64) f32, key_padding_mask (2,2048)
bool.  Reference: GQA attention with additive -10000 padding bias and a causal
mask shifted by the per-batch valid key count sk, softmax over keys.

Key observations used here:
  * Every padded key position is also causal-masked, so only the shifted-causal
    structure matters.  With u := q_idx - (2048 - sk) the valid region is the
    causal triangle u >= k over the first sk keys; the shift is applied on the
    HOST when laying out Q^T per core, so the device program is a static causal
    flash-attention kernel.
  * Rows q_idx < 2048 - sk have no valid key: reference softmaxes equal
    -10000s -> uniform weights -> output = mean over all 2048 v rows.  Pure
    host-side fixup.
  * exp without max-subtraction is safe (|score*0.125| <~ 8); the softmax
    denominator comes from a ones-column appended to V (PV matmul yields
    [num | den]); the division happens on host.

Device program (per core, 4 head-instances = 2 heads x 2 batches):
  u-chunk-outer loop (512-wide output chunks, [65,512] single-bank fp32 PSUM
  accumulators, two chunks live so TWO INSTANCES are processed interleaved --
  the PE always has an independent stream while the other instance's exp
  catches up).  128-key score blocks are processed in PAIRS sharing one
  u-stream window: the PE row-tiles the two QK matmuls at row groups
  0-63 / 64-127 (K^T and Q^T are host-duplicated into both SBUF partition
  halves), producing TWO full [128 x w] score strips in w cycles -- 2x QK
  throughput vs one matmul.  Per strip: exp on ScalarE (or a Schraudolph
  fast-exp2 on VectorE for a third of the strips, balancing the two
  engines), triangle mask on the diagonal block, then
  [num|den] += V'.T @ P^T (PV emission delayed one pair so the tensor queue
  never stalls on the activation).  Six single-bank score tiles give the PE
  ~3 strips of dependency runway toward the warm HAM clock.
  Startup: critical-path-first DMAs and exp-table preload during the initial
  DMA wait.  Output DMA'd as fp16.
"""

import os
import ml_dtypes
import numpy as np

F16 = np.float16

import concourse.bass as bass
import concourse.mybir as mybir
import concourse.tile as tile
from concourse import bacc
from concourse.bass_utils import run_bass_kernel_spmd

B, SQ, SK, H, HK, D = 2, 2048, 2048, 16, 4, 64
NCORES = 8
P = 128
FP = mybir.dt.float32
FR = mybir.dt.float16
I16 = mybir.dt.int16
CHW = 512   # u-chunk width (1 PSUM bank of fp32)

# Schraudolph fast-exp2 constants for fp16 bit-pattern exp(0.125*s):
# i16 = trunc(s*EXPA + EXPB); bitcast fp16.  max rel err ~3.0%, mean +1%.
EXPA = 0.125 * 1.4426950408889634 * 1024.0
EXPB = 15315.5
DVE_EXP_PERIOD = 3  # every 3rd strip exp'd on VectorE (0 = disabled)
WARMUP_MM = 0       # junk matmuls at t0 to release the PE HAM clock gate

LAST_EXEC_NS = None


def _ceil_div(a, b):
    return -(-a // b)


def _build_program(sks):
    """Build + compile the SPMD program for per-batch valid key counts sks."""
    nc = bacc.Bacc("TRN2", target_bir_lowering=False, debug=False,
                   num_devices=NCORES)

    # q/k duplicated into both partition halves on host: [128, S]
    qT_d = nc.dram_tensor("qT", [4, P, SQ], FR, kind="ExternalInput").ap()
    kT_d = nc.dram_tensor("kT", [B, P, SK], FR, kind="ExternalInput").ap()
    vp_d = nc.dram_tensor("vp", [B, P, (SK // P) * 65], FR,
                          kind="ExternalInput").ap()
    tri_d = nc.dram_tensor("tri", [P, P], FR, kind="ExternalInput").ap()
    out_d = nc.dram_tensor("outT", [4, 65, SQ], FR, kind="ExternalOutput").ap()

    EXP = mybir.ActivationFunctionType.Exp
    MUL = mybir.AluOpType.mult
    ADD = mybir.AluOpType.add

    with tile.TileContext(nc) as tc:
        with (
            tc.tile_pool(name="const", bufs=1) as cpool,
            tc.tile_pool(name="kv", bufs=1) as kvpool,
            tc.tile_pool(name="qin", bufs=4) as qpool,
            tc.tile_pool(name="pt", bufs=10) as ppool,
            tc.tile_pool(name="oc", bufs=3) as opool,
            tc.tile_pool(name="ps", bufs=6, space="PSUM") as spool,
            tc.tile_pool(name="pa", bufs=2, space="PSUM") as apool,
        ):
            kT_sb = []
            vp_sb = []
            for b in range(B):
                kt_t = kvpool.tile([P, SK], FR, name=f"kT{b}", tag=f"kT{b}")
                kT_sb.append(kt_t)
                vp_t = kvpool.tile([P, (SK // P) * 65], FR, name=f"vp{b}",
                                   tag=f"vp{b}")
                vp_sb.append(vp_t)
            tri_sb = cpool.tile([P, P], FR, name="tri_sb")
            q_sb = [qpool.tile([P, SQ], FR, name=f"q{j}", tag=f"q{j}")
                    for j in range(4)]

            # --- input DMAs, critical path first; the two transfers the
            # first matmul pair needs go on the otherwise-idle Activation
            # HWDGE ring so they complete in parallel with the Sync ring ---
            nc.scalar.dma_start(q_sb[0][:, 0:512], qT_d[0][:, 0:512])
            nc.scalar.dma_start(kT_sb[0][:, 0:256], kT_d[0][:, 0:256])
            # exp-table preload during the DMA wait
            dmy = cpool.tile([P, 32], FR, name="dmy")
            nc.vector.memset(dmy[:, 0:16], 0.0)
            nc.scalar.activation(dmy[:, 16:32], dmy[:, 0:16], EXP, scale=1.0)
            nc.sync.dma_start(tri_sb[:], tri_d[:])
            nc.sync.dma_start(q_sb[1][:, 0:512], qT_d[1][:, 0:512])
            nc.sync.dma_start(vp_sb[0][:, 0:130], vp_d[0][:, 0:130])
            nc.sync.dma_start(q_sb[0][:, 512:1024], qT_d[0][:, 512:1024])
            nc.sync.dma_start(kT_sb[0][:, 256:1024], kT_d[0][:, 256:1024])
            nc.sync.dma_start(q_sb[1][:, 512:1024], qT_d[1][:, 512:1024])
            nc.sync.dma_start(vp_sb[0][:, 130:1040], vp_d[0][:, 130:1040])
            nc.sync.dma_start(q_sb[0][:, 1024:2048], qT_d[0][:, 1024:2048])
            nc.sync.dma_start(q_sb[1][:, 1024:2048], qT_d[1][:, 1024:2048])
            nc.sync.dma_start(kT_sb[0][:, 1024:2048], kT_d[0][:, 1024:2048])
            nc.sync.dma_start(q_sb[2][:], qT_d[2])
            nc.sync.dma_start(kT_sb[1][:], kT_d[1])
            nc.sync.dma_start(vp_sb[1][:], vp_d[1])
            nc.sync.dma_start(q_sb[3][:], qT_d[3])

            # --- PE clock warm-up: junk matmuls as soon as q piece 0 lands ---
            if WARMUP_MM:
                wps = spool.tile([P, 512], FP, name="ps", tag="ps")
                for i in range(WARMUP_MM):
                    nc.tensor.matmul(
                        wps[:, 0:512], lhsT=q_sb[0][0:64, 0:128],
                        rhs=q_sb[0][0:64, 0:512],
                        start=True, stop=True, skip_group_check=True)

            nstrip = [0]  # round-robin counter for DVE exp offload

            def inst_work(j):
                b = 0 if j < 2 else 1
                U = sks[b]
                KT = _ceil_div(U, P)
                NCH = _ceil_div(U, CHW)
                for c in range(NCH):
                    cs = CHW * c
                    ce = min(U, CHW * (c + 1))
                    acc = apool.tile([65, CHW], FP, name="acc", tag="acc")
                    kt_hi = min(KT, _ceil_div(ce, P))
                    stop = kt_hi - 1

                    # delayed PV work: (kt, kw, m0, e0, pt_ap)
                    pending = []

                    def flush_pv():
                        for (fkt, fkw, fm0, fe0, fpt) in pending:
                            nc.tensor.matmul(
                                acc[:, fe0 - cs:ce - cs],
                                lhsT=vp_sb[b][0:fkw,
                                              65 * fkt:65 * (fkt + 1)],
                                rhs=fpt[0:fkw, fe0 - fm0:ce - fm0],
                                start=(fkt == 0), stop=(fkt == stop),
                                skip_group_check=True,
                            )
                        pending.clear()

                    for t in range(_ceil_div(kt_hi, 2)):
                        blocks = [2 * t]
                        if 2 * t + 1 < kt_hi:
                            blocks.append(2 * t + 1)
                        k0A = P * 2 * t
                        m0 = max(cs, k0A)  # shared stream window [m0, ce)
                        # QK: both blocks stream the window, row-tiled at PE
                        # row groups 0-63 / 64-127 (concurrent)
                        ptile = []
                        for bi, kt in enumerate(blocks):
                            k0 = P * kt
                            kw = min(P, U - k0)
                            hf = 64 * bi
                            ps = spool.tile([P, CHW], FP, name="ps",
                                            tag="ps")
                            ptile.append(ps)
                            nc.tensor.matmul(
                                ps[0:kw, 0:ce - m0],
                                lhsT=kT_sb[b][hf:hf + 64, k0:k0 + kw],
                                rhs=q_sb[j][hf:hf + 64, m0:ce],
                                start=True, stop=True,
                                skip_group_check=True,
                            )

                        newpv = []
                        for bi, kt in enumerate(blocks):
                            k0 = P * kt
                            kw = min(P, U - k0)
                            e0 = max(cs, k0)  # strip's own valid start
                            ps = ptile[bi]
                            use_dve = (DVE_EXP_PERIOD and
                                       nstrip[0] % DVE_EXP_PERIOD ==
                                       DVE_EXP_PERIOD - 1)
                            nstrip[0] += 1
                            if use_dve:
                                pti = ppool.tile([P, CHW], I16,
                                                 name="pt", tag="pt")
                                nc.vector.tensor_scalar(
                                    pti[0:kw, e0 - m0:ce - m0],
                                    ps[0:kw, e0 - m0:ce - m0],
                                    EXPA, EXPB, MUL, ADD)
                                pt = pti.bitcast(FR)
                            else:
                                pt = ppool.tile([P, CHW], FR,
                                                name="pt", tag="pt")
                                nc.scalar.activation(
                                    pt[0:kw, e0 - m0:ce - m0],
                                    ps[0:kw, e0 - m0:ce - m0],
                                    EXP, scale=0.125)
                            # causal triangle mask on the diagonal block
                            d0 = max(k0, e0)
                            d1 = min(k0 + P, ce)
                            if k0 >= cs and d0 < d1:
                                nc.vector.tensor_mul(
                                    pt[0:kw, d0 - m0:d1 - m0],
                                    pt[0:kw, d0 - m0:d1 - m0],
                                    tri_sb[0:kw, d0 - k0:d1 - k0])
                            newpv.append((kt, kw, m0, e0, pt))

                        flush_pv()
                        pending.extend(newpv)
                        yield

                    flush_pv()
                    cw = ce - cs
                    oc = opool.tile([65, CHW], FR, name="oc", tag="oc")
                    nc.vector.tensor_copy(oc[:, 0:cw], acc[:, 0:cw])
                    nc.sync.dma_start(out_d[j, :, cs:cs + cw], oc[:, 0:cw])
                    yield

            # interleave two instances at a time so the PE always has an
            # independent stream to chew on while the other's exp catches up
            for pair in ((0, 1), (2, 3)):
                gens = [inst_work(pair[0]), inst_work(pair[1])]
                done = [False, False]
                i = 0
                while not all(done):
                    if not done[i]:
                        try:
                            next(gens[i])
                        except StopIteration:
                            done[i] = True
                    i ^= 1

    nc.compile()
    return nc


_prog_cache = {}


def _get_program(sks):
    if sks not in _prog_cache:
        _prog_cache[sks] = _build_program(sks)
    return _prog_cache[sks]


def kernel(q, kv, key_padding_mask):
    global LAST_EXEC_NS
    q = np.asarray(q, dtype=np.float32)
    kv = np.asarray(kv, dtype=np.float32)
    mask = np.asarray(key_padding_mask)

    sk = mask.sum(axis=1).astype(np.int64)  # (B,) valid key counts
    c = (SQ - sk).astype(np.int64)
    prog = _get_program((int(sk[0]), int(sk[1])))

    k_all = kv[:, :, 0]  # (B, SK, HK, D)
    v_all = kv[:, :, 1]

    tri = (np.arange(P)[None, :] >= np.arange(P)[:, None]).astype(np.float32)

    kT_by_g = {}
    vp_by_g = {}
    for g in range(HK):
        kT = k_all[:, :, g, :].transpose(0, 2, 1)  # (B, D, SK)
        kT_by_g[g] = np.ascontiguousarray(
            np.concatenate([kT, kT], axis=1))  # (B, 128, SK) duplicated
        vpz = np.ones((B, SK, 65), dtype=np.float32)
        vpz[:, :, :64] = v_all[:, :, g, :]
        vp = vpz.reshape(B, SK // P, P, 65).transpose(0, 2, 1, 3)
        vp_by_g[g] = np.ascontiguousarray(vp.reshape(B, P, (SK // P) * 65))

    def core_instances(core):
        g = core // 2
        hp = core % 2
        h0 = 4 * g + 2 * hp
        return g, [(0, h0), (0, h0 + 1), (1, h0), (1, h0 + 1)]

    in_maps = []
    for core in range(NCORES):
        g, insts = core_instances(core)
        qT = np.zeros((4, P, SQ), dtype=np.float32)
        for jj, (b, h) in enumerate(insts):
            U = int(sk[b])
            qh = q[b, c[b]:, h, :].T  # (64, U)
            qT[jj, 0:64, :U] = qh
            qT[jj, 64:128, :U] = qh
        in_maps.append({
            "qT": qT.astype(F16),
            "kT": kT_by_g[g].astype(F16),
            "vp": vp_by_g[g].astype(F16),
            "tri": tri.astype(F16),
        })

    trace = bool(os.environ.get("BASS_KERNEL_TRACE"))
    res = run_bass_kernel_spmd(prog, in_maps, list(range(NCORES)),
                               trace=trace)
    LAST_EXEC_NS = res.exec_time_ns

    out = np.empty((B, SQ, H, D), dtype=np.float32)
    # fully-masked rows: uniform softmax over all SK keys -> mean of v
    vmean = v_all.mean(axis=1)  # (B, HK, D)
    for b in range(B):
        if c[b] > 0:
            for g in range(HK):
                for h in range(4 * g, 4 * g + 4):
                    out[b, :c[b], h, :] = vmean[b, g]

    for core in range(NCORES):
        g, insts = core_instances(core)
        o = np.asarray(res.results[core]["outT"], dtype=np.float32)
        for jj, (b, h) in enumerate(insts):
            U = int(sk[b])
            num = o[jj, :64, :U]
            den = o[jj, 64, :U]
            out[b, c[b]:, h, :] = (num / den[None, :]).T

    return out
